# revision 32
# baseline (speedup 1.0000x reference)
"""Trainium2 Bass kernel for CentroidDistance (retrieval_knn).

reference math:
  d2[n,k]  = ||x_n||^2 + ||c_k||^2 - 2 x_n.c_k
  dist     = sqrt(max(d2, 1e-12))
  node_out = dist * mask            # [1, N, K]
  graph    = sum_n(node_out) / sum(mask)   # [1, K]

Sharding: data-parallel over nodes. node_repr/mask split into 8 row-shards
(4096 rows each); the [K, D] centroid table is replicated on every core.
Each core computes its [4096, 2048] slab of node_out plus a column-sum
partial; the host concatenates slabs and reduces the 8 partials.

Per-core device pipeline ([n, k] output layout, n on PSUM partitions):
  PE  : psum[n,k] = sum_d xT[d,n] * (-2 c)T[d,k]     fp8e4 DoubleRow, fp32 accum
  DVE : s = (psum + x2[n]) + c2bcast[k]              one scalar_tensor_tensor
  ACT : o = Sqrt(mask[n]^2 * s)                      == mask * dist  (mask >= 0)
  PE  : csum[1,k] += ones[128].T @ o                 float32r, 1 cyc/row
  DMA : o -> out[n,k]

The Gram term is the only fp8 quantity; x2/c2 ride in fp32, so the d2 error
is ~|2 delta(x.c)| ~ 1e-3 relative on dist, well inside the 2e-2 gate.

x2 = ||x_n||^2 and c2 = ||c_k||^2 ride in from the host in exact fp32; the
-2 scale is folded into the replicated centroid operand.
"""

import sys
import types

sys.path.insert(0, "/opt/trn_rl_repo")

import numpy as np
import ml_dtypes

import concourse.bass as bass
from concourse import bacc
import concourse.tile as tile
import concourse.mybir as mybir

N, K, D = 32768, 2048, 512
N_CORES = 8
NS = N // N_CORES          # 4096 rows per core
P = 128                    # partitions
N_TILES = NS // P          # 32 row-tiles per core
D_CHUNKS = D // P          # 4 contraction chunks (DoubleRow consumes them in pairs)
KH = 1024                  # k-half processed per psum tile
COLSUM_LAG = 4             # (t, kh) units between ACT out and its colsum MM

_PROGRAM = None


def _register_ntff_hook():
    """Restore the axon NTFF profile hook that the boot path skips when
    antenv lacks axon_hooks. Needed only for trace=True runs."""
    if "antenv.axon_hooks" in sys.modules:
        return
    try:
        from trn_agent_boot.trn_boot import _ntff_profile_via_ctypes

        hook = _ntff_profile_via_ctypes("/opt/axon/libaxon_pjrt.so")
    except Exception:
        hook = None
    m = types.ModuleType("antenv.axon_hooks")
    m.get_axon_ntff_profile_hook = lambda: hook
    sys.modules["antenv.axon_hooks"] = m


_register_ntff_hook()


def _build_program() -> bacc.Bacc:
    import os
    use_dr = os.environ.get("K_NO_DR", "0") != "1"      # DoubleRow on/off
    use_cs = os.environ.get("K_NO_CS", "0") != "1"      # colsum MMs on/off
    nc = bacc.Bacc("TRN2", target_bir_lowering=False, debug=False)
    f32, f32r = mybir.dt.float32, mybir.dt.float32r
    fp8 = mybir.dt.float8e4

    xT_d = nc.dram_tensor("xT", [D, NS], fp8, kind="ExternalInput")
    # centroid operand pair-interleaved for DoubleRow: [cpair, p, k, i] where
    # logical d = 256*cpair + 128*i + p. Adjacent (i) pairs let the PE stream
    # 2 fp8/cycle; split pairs would halve matmul throughput.
    cT2_d = nc.dram_tensor("cT2", [D_CHUNKS // 2, P, K, 2], fp8, kind="ExternalInput")
    c2_d = nc.dram_tensor("c2r", [1, K], f32, kind="ExternalInput")
    x2_d = nc.dram_tensor("x2c", [P, N_TILES], f32, kind="ExternalInput")
    m2_d = nc.dram_tensor("m2c", [P, N_TILES], f32, kind="ExternalInput")
    ones_d = nc.dram_tensor("onesr", [P, 1], f32r, kind="ExternalInput")
    out_d = nc.dram_tensor("out", [NS, K], f32, kind="ExternalOutput")
    cs_d = nc.dram_tensor("csum", [1, K], f32, kind="ExternalOutput")

    # dram views with the 128-partition contraction chunking exposed
    xT_v = xT_d.ap().rearrange("(c p) n -> p c n", p=P)    # [128, 4, NS]
    cT2_v = cT2_d.ap().rearrange("c p k i -> p c (k i)")   # [128, 2, K*2]

    with tile.TileContext(nc) as tc:
        with (
            tc.tile_pool(name="consts", bufs=1) as consts,
            tc.tile_pool(name="xin", bufs=1) as xin,
            tc.tile_pool(name="work", bufs=3) as work,
            tc.tile_pool(name="ovec", bufs=6) as ovec,
            tc.tile_pool(name="mm", bufs=2, space="PSUM") as mm_pool,
            tc.tile_pool(name="acc", bufs=1, space="PSUM") as acc_pool,
        ):
            # Warm the sqrt spline tables (~2.7us) while input DMAs run, off
            # the first real ACTIVATE's critical path.
            warm = consts.tile([1, 1], f32, name="warm")
            nc.vector.memset(warm[:], 1.0)
            warm2 = consts.tile([1, 1], f32, name="warm2")
            nc.scalar.activation(warm2[:], warm[:], mybir.ActivationFunctionType.Sqrt)

            # ---- inputs. x is only 2MB in fp8, so it lives in SBUF whole
            # (16KB/partition), loaded per d-chunk with full-width 4KB DMA
            # lines. xt/ct go through gpsimd's SWDGE queues so they never
            # serialize behind the output-DMA issue stream on Sync; the
            # first matmul's operands (chunks 0-1, ct kh=0) issue first. ----
            xall = xin.tile([P, D_CHUNKS, NS], fp8, name="xall")
            for c in range(2):
                nc.gpsimd.dma_start(xall[:, c, :], xT_v[:, c, :])
            cth = []
            for kh in range(K // KH):
                c_half = consts.tile([P, D_CHUNKS // 2, KH * 2], fp8, name=f"ct{kh}")
                nc.gpsimd.dma_start(
                    c_half[:], cT2_v[:, :, kh * KH * 2 : (kh + 1) * KH * 2]
                )
                cth.append(c_half)
                nc.gpsimd.dma_start(xall[:, 2 + kh, :], xT_v[:, 2 + kh, :])

            c2b = consts.tile([P, K], f32, name="c2b")
            nc.sync.dma_start(c2b[:], c2_d.ap().to_broadcast([P, K]))
            x2c = consts.tile([P, N_TILES], f32, name="x2c")
            nc.sync.dma_start(x2c[:], x2_d.ap())
            m2c = consts.tile([P, N_TILES], f32, name="m2c")
            nc.sync.dma_start(m2c[:], m2_d.ap())
            ones = consts.tile([P, 1], f32r, name="ones")
            nc.sync.dma_start(ones[:], ones_d.ap())

            csum_ps = acc_pool.tile([1, K], f32, name="csum_ps")

            n_units = N_TILES * (K // KH)
            pending = []  # deferred colsum emitters, keyed by unit index

            def emit_colsum(o_t, kh, unit):
                if not use_cs:
                    return
                for kb in range(KH // 512):
                    nc.tensor.matmul(
                        csum_ps[0:1, kh * KH + kb * 512 : kh * KH + (kb + 1) * 512],
                        ones[:],
                        o_t[:, kb * 512 : (kb + 1) * 512],
                        start=(unit < K // KH),
                        stop=(unit >= n_units - K // KH),
                    )

            unit = 0
            for t in range(N_TILES):
                if True:
                    for kh in range(K // KH):
                        ps = mm_pool.tile([P, KH], mybir.dt.float32, name="ps")
                        for cp in range(D_CHUNKS // 2):
                            lhsT = xall[:, 2 * cp : 2 * cp + 2, t * P : (t + 1) * P]
                            for kb in range(KH // 512):
                                rhs = cth[kh][
                                    :, cp, kb * 1024 : (kb + 1) * 1024
                                ].rearrange("p (j i) -> p i j", i=2)
                                nc.tensor.matmul(
                                    ps[:, kb * 512 : (kb + 1) * 512],
                                    lhsT,
                                    rhs,
                                    start=(cp == 0),
                                    stop=(cp == D_CHUNKS // 2 - 1),
                                    perf_mode=mybir.MatmulPerfMode.DoubleRow,
                                )
                        s_t = work.tile([P, KH], mybir.dt.float32, name="s_t")
                        nc.vector.scalar_tensor_tensor(
                            s_t[:],
                            ps[:],
                            x2c[:, t : t + 1],
                            c2b[:, kh * KH : (kh + 1) * KH],
                            op0=mybir.AluOpType.add,
                            op1=mybir.AluOpType.add,
                        )
                        # float32r so the colsum matmul can consume it at
                        # 1 cyc/row; bits are fp32 rounded to TF32 precision
                        # (~5e-4 rel on dist), DMA'd out via bitcast.
                        o_t = ovec.tile([P, KH], mybir.dt.float32r, name="o_t")
                        nc.scalar.activation(
                            o_t[:],
                            s_t[:],
                            mybir.ActivationFunctionType.Sqrt,
                            bias=0.0,
                            scale=m2c[:, t : t + 1],
                        )
                        nc.sync.dma_start(
                            out_d.ap()[t * P : (t + 1) * P, kh * KH : (kh + 1) * KH],
                            o_t[:].bitcast(mybir.dt.float32),
                        )
                        pending.append((o_t, kh, unit))
                        unit += 1
                        if len(pending) > COLSUM_LAG:
                            emit_colsum(*pending.pop(0))
            for args in pending:
                emit_colsum(*args)

            cs_sb = consts.tile([1, K], f32, name="cs_sb")
            if use_cs:
                nc.vector.tensor_copy(cs_sb[:], csum_ps[:])
            else:
                nc.vector.memset(cs_sb[:], 0.0)
            nc.sync.dma_start(cs_d.ap(), cs_sb[:])

    nc.compile()
    return nc


def _get_program() -> bacc.Bacc:
    global _PROGRAM
    if _PROGRAM is None:
        _PROGRAM = _build_program()
    return _PROGRAM


def _prep_in_maps(node_repr, mask, centroid_weight):
    x = np.ascontiguousarray(np.asarray(node_repr, dtype=np.float32))
    msk = np.asarray(mask, dtype=np.float32).reshape(N)
    c = np.asarray(centroid_weight, dtype=np.float32)

    fp8_np = mybir.dt.np(mybir.dt.float8e4)
    # [cpair, p, k, i]: d = 256*cpair + 128*i + p, DoubleRow pair-interleaved
    cT2 = (-2.0 * c).T.reshape(2, 2, P, K).transpose(0, 2, 3, 1)
    cT2 = np.ascontiguousarray(cT2).astype(fp8_np)
    c2 = np.einsum("kd,kd->k", c.astype(np.float64), c.astype(np.float64))
    c2 = c2.astype(np.float32)[None, :]                                  # [1, K]
    x2 = np.einsum("nd,nd->n", x.astype(np.float64), x.astype(np.float64))
    x2 = x2.astype(np.float32)                                           # [N]
    m2 = (msk.astype(np.float64) ** 2).astype(np.float32)                # [N]

    in_maps = []
    for i in range(N_CORES):
        sl = slice(i * NS, (i + 1) * NS)
        xT = np.ascontiguousarray(x[sl].T).astype(fp8_np)                # [D, NS]
        x2c = np.ascontiguousarray(x2[sl].reshape(N_TILES, P).T)         # [P, 32]
        m2c = np.ascontiguousarray(m2[sl].reshape(N_TILES, P).T)         # [P, 32]
        in_maps.append({"xT": xT, "cT2": cT2, "c2r": c2, "x2c": x2c, "m2c": m2c,
                        "onesr": np.ones((P, 1), dtype=np.float32)})
    return in_maps


def run(node_repr, mask, centroid_weight, trace=False):
    """Run the device kernel; returns ((graph, node), BassKernelResults)."""
    from concourse.bass_utils import run_bass_kernel_spmd

    in_maps = _prep_in_maps(node_repr, mask, centroid_weight)
    nc = _get_program()
    res = run_bass_kernel_spmd(nc, in_maps, list(range(N_CORES)), trace=trace)

    node_out = np.empty((1, N, K), dtype=np.float32)
    csum = np.zeros(K, dtype=np.float64)
    for i in range(N_CORES):
        node_out[0, i * NS : (i + 1) * NS] = res.results[i]["out"]
        csum += res.results[i]["csum"][0].astype(np.float64)
    msum = float(np.asarray(mask, dtype=np.float64).sum())
    graph = (csum / msum).astype(np.float32)[None, :]  # [1, K]
    return (graph, node_out), res


def kernel(node_repr, mask, centroid_weight):
    (graph, node_out), _ = run(node_repr, mask, centroid_weight, trace=False)
    return graph, node_out


# revision 35
# speedup vs baseline: 1.2263x; 1.2263x over previous
"""Trainium2 Bass kernel for CentroidDistance (retrieval_knn).

reference math:
  d2[n,k]  = ||x_n||^2 + ||c_k||^2 - 2 x_n.c_k
  dist     = sqrt(max(d2, 1e-12))
  node_out = dist * mask            # [1, N, K]
  graph    = sum_n(node_out) / sum(mask)   # [1, K]

Sharding: data-parallel over nodes. node_repr/mask split into 8 row-shards
(4096 rows each); the [K, D] centroid table is replicated on every core.
Each core computes its [4096, 2048] slab of node_out plus a column-sum
partial; the host concatenates slabs and reduces the 8 partials.

Per-core device pipeline ([n, k] output layout, n on PSUM partitions):
  PE  : psum[n,k] = sum_d xT[d,n] * (-2 c)T[d,k]     fp8e4 DoubleRow, fp32 accum
  DVE : s = (psum + x2[n]) + c2bcast[k]              one scalar_tensor_tensor
  ACT : o = Sqrt(mask[n]^2 * s)                      == mask * dist  (mask >= 0)
  PE  : csum[1,k] += ones[128].T @ o                 float32r, 1 cyc/row
  DMA : o -> out[n,k]

The Gram term is the only fp8 quantity; x2/c2 ride in fp32, so the d2 error
is ~|2 delta(x.c)| ~ 1e-3 relative on dist, well inside the 2e-2 gate.

x2 = ||x_n||^2 and c2 = ||c_k||^2 ride in from the host in exact fp32; the
-2 scale is folded into the replicated centroid operand.
"""

import sys
import types

sys.path.insert(0, "/opt/trn_rl_repo")

import numpy as np
import ml_dtypes

import concourse.bass as bass
from concourse import bacc
import concourse.tile as tile
import concourse.mybir as mybir

N, K, D = 32768, 2048, 512
N_CORES = 8
NS = N // N_CORES          # 4096 rows per core
P = 128                    # partitions
N_TILES = NS // P          # 32 row-tiles per core
D_CHUNKS = D // P          # 4 contraction chunks (DoubleRow consumes them in pairs)
KH = 1024                  # k-half processed per psum tile
COLSUM_LAG = 4             # (t, kh) units between ACT out and its colsum MM

_PROGRAM = None


def _register_ntff_hook():
    """Restore the axon NTFF profile hook that the boot path skips when
    antenv lacks axon_hooks. Needed only for trace=True runs."""
    if "antenv.axon_hooks" in sys.modules:
        return
    try:
        from trn_agent_boot.trn_boot import _ntff_profile_via_ctypes

        hook = _ntff_profile_via_ctypes("/opt/axon/libaxon_pjrt.so")
    except Exception:
        hook = None
    m = types.ModuleType("antenv.axon_hooks")
    m.get_axon_ntff_profile_hook = lambda: hook
    sys.modules["antenv.axon_hooks"] = m


_register_ntff_hook()


def _build_program() -> bacc.Bacc:
    import os
    use_dr = os.environ.get("K_NO_DR", "0") != "1"      # DoubleRow on/off
    use_cs = os.environ.get("K_NO_CS", "0") != "1"      # colsum MMs on/off
    nc = bacc.Bacc("TRN2", target_bir_lowering=False, debug=False)
    f32, f32r = mybir.dt.float32, mybir.dt.float32r
    fp8 = mybir.dt.float8e4

    xT_d = nc.dram_tensor("xT", [D, NS], fp8, kind="ExternalInput")
    # centroid operand pair-interleaved for DoubleRow: [cpair, p, k, i] where
    # logical d = 256*cpair + 128*i + p. Adjacent (i) pairs let the PE stream
    # 2 fp8/cycle; split pairs would halve matmul throughput.
    cT2_d = nc.dram_tensor("cT2", [D_CHUNKS // 2, P, K, 2], fp8, kind="ExternalInput")
    c2_d = nc.dram_tensor("c2r", [1, K], f32, kind="ExternalInput")
    x2_d = nc.dram_tensor("x2c", [P, N_TILES], f32, kind="ExternalInput")
    m2_d = nc.dram_tensor("m2c", [P, N_TILES], f32, kind="ExternalInput")
    ones_d = nc.dram_tensor("onesr", [P, 1], f32r, kind="ExternalInput")
    out_d = nc.dram_tensor("out", [NS, K], f32, kind="ExternalOutput")
    cs_d = nc.dram_tensor("csum", [1, K], f32, kind="ExternalOutput")

    # dram views with the 128-partition contraction chunking exposed
    xT_v = xT_d.ap().rearrange("(c p) n -> p c n", p=P)    # [128, 4, NS]
    cT2_v = cT2_d.ap().rearrange("c p k i -> p c (k i)")   # [128, 2, K*2]

    with tile.TileContext(nc) as tc:
        with (
            tc.tile_pool(name="consts", bufs=1) as consts,
            tc.tile_pool(name="xin", bufs=1) as xin,
            tc.tile_pool(name="work", bufs=3) as work,
            tc.tile_pool(name="ovec", bufs=6) as ovec,
            tc.tile_pool(name="mm", bufs=2, space="PSUM") as mm_pool,
            tc.tile_pool(name="acc", bufs=1, space="PSUM") as acc_pool,
        ):
            # Warm the sqrt spline tables (~2.7us) while input DMAs run, off
            # the first real ACTIVATE's critical path.
            warm = consts.tile([1, 1], f32, name="warm")
            nc.vector.memset(warm[:], 1.0)
            warm2 = consts.tile([1, 1], f32, name="warm2")
            nc.scalar.activation(warm2[:], warm[:], mybir.ActivationFunctionType.Sqrt)

            # ---- inputs. x is only 2MB in fp8, so it lives in SBUF whole
            # (16KB/partition), loaded per d-chunk with full-width 4KB DMA
            # lines. xt/ct go through gpsimd's SWDGE queues so they never
            # serialize behind the output-DMA issue stream on Sync; the
            # first matmul's operands (chunks 0-1, ct kh=0) issue first. ----
            # n-sliced loads: the first matmul only needs x rows 0-511 plus
            # ct kh=0, so those issue first and compute starts ~8us earlier
            # than a whole-x load would allow.
            xall = xin.tile([P, D_CHUNKS, NS], fp8, name="xall")
            nc.gpsimd.dma_start(xall[:, :, 0:512], xT_v[:, :, 0:512])
            cth = []
            for kh in range(K // KH):
                c_half = consts.tile([P, D_CHUNKS // 2, KH * 2], fp8, name=f"ct{kh}")
                nc.gpsimd.dma_start(
                    c_half[:], cT2_v[:, :, kh * KH * 2 : (kh + 1) * KH * 2]
                )
                cth.append(c_half)
                nc.gpsimd.dma_start(
                    xall[:, :, 512 * (kh + 1) : 512 * (kh + 2)],
                    xT_v[:, :, 512 * (kh + 1) : 512 * (kh + 2)],
                )
            for n0 in range(1536, NS, 1280):
                n1 = min(n0 + 1280, NS)
                nc.gpsimd.dma_start(xall[:, :, n0:n1], xT_v[:, :, n0:n1])

            c2b = consts.tile([P, K], f32, name="c2b")
            nc.sync.dma_start(c2b[:], c2_d.ap().to_broadcast([P, K]))
            x2c = consts.tile([P, N_TILES], f32, name="x2c")
            nc.sync.dma_start(x2c[:], x2_d.ap())
            m2c = consts.tile([P, N_TILES], f32, name="m2c")
            nc.sync.dma_start(m2c[:], m2_d.ap())
            ones = consts.tile([P, 1], f32r, name="ones")
            nc.sync.dma_start(ones[:], ones_d.ap())

            csum_ps = acc_pool.tile([1, K], f32, name="csum_ps")

            n_units = N_TILES * (K // KH)
            pending = []  # deferred colsum emitters, keyed by unit index

            def emit_colsum(o_t, kh, unit):
                if not use_cs:
                    return
                for kb in range(KH // 512):
                    nc.tensor.matmul(
                        csum_ps[0:1, kh * KH + kb * 512 : kh * KH + (kb + 1) * 512],
                        ones[:],
                        o_t[:, kb * 512 : (kb + 1) * 512],
                        start=(unit < K // KH),
                        stop=(unit >= n_units - K // KH),
                    )

            unit = 0
            for t in range(N_TILES):
                if True:
                    for kh in range(K // KH):
                        ps = mm_pool.tile([P, KH], mybir.dt.float32, name="ps")
                        for cp in range(D_CHUNKS // 2):
                            lhsT = xall[:, 2 * cp : 2 * cp + 2, t * P : (t + 1) * P]
                            for kb in range(KH // 512):
                                rhs = cth[kh][
                                    :, cp, kb * 1024 : (kb + 1) * 1024
                                ].rearrange("p (j i) -> p i j", i=2)
                                nc.tensor.matmul(
                                    ps[:, kb * 512 : (kb + 1) * 512],
                                    lhsT,
                                    rhs,
                                    start=(cp == 0),
                                    stop=(cp == D_CHUNKS // 2 - 1),
                                    perf_mode=mybir.MatmulPerfMode.DoubleRow,
                                )
                        s_t = work.tile([P, KH], mybir.dt.float32, name="s_t")
                        nc.vector.scalar_tensor_tensor(
                            s_t[:],
                            ps[:],
                            x2c[:, t : t + 1],
                            c2b[:, kh * KH : (kh + 1) * KH],
                            op0=mybir.AluOpType.add,
                            op1=mybir.AluOpType.add,
                        )
                        # float32r so the colsum matmul can consume it at
                        # 1 cyc/row; bits are fp32 rounded to TF32 precision
                        # (~5e-4 rel on dist), DMA'd out via bitcast.
                        o_t = ovec.tile([P, KH], mybir.dt.float32r, name="o_t")
                        nc.scalar.activation(
                            o_t[:],
                            s_t[:],
                            mybir.ActivationFunctionType.Sqrt,
                            bias=0.0,
                            scale=m2c[:, t : t + 1],
                        )
                        nc.sync.dma_start(
                            out_d.ap()[t * P : (t + 1) * P, kh * KH : (kh + 1) * KH],
                            o_t[:].bitcast(mybir.dt.float32),
                        )
                        pending.append((o_t, kh, unit))
                        unit += 1
                        if len(pending) > COLSUM_LAG:
                            emit_colsum(*pending.pop(0))
            for args in pending:
                emit_colsum(*args)

            # split tail drain: DVE and ACT each evacuate one k-half of the
            # colsum accumulator in parallel
            cs_sb = consts.tile([1, K], f32, name="cs_sb")
            if use_cs:
                nc.vector.tensor_copy(cs_sb[:, 0:KH], csum_ps[:, 0:KH])
                nc.scalar.copy(cs_sb[:, KH:K], csum_ps[:, KH:K])
            else:
                nc.vector.memset(cs_sb[:], 0.0)
            nc.sync.dma_start(cs_d.ap(), cs_sb[:])

    nc.compile()
    return nc


def _get_program() -> bacc.Bacc:
    global _PROGRAM
    if _PROGRAM is None:
        _PROGRAM = _build_program()
    return _PROGRAM


def _prep_in_maps(node_repr, mask, centroid_weight):
    x = np.ascontiguousarray(np.asarray(node_repr, dtype=np.float32))
    msk = np.asarray(mask, dtype=np.float32).reshape(N)
    c = np.asarray(centroid_weight, dtype=np.float32)

    fp8_np = mybir.dt.np(mybir.dt.float8e4)
    # [cpair, p, k, i]: d = 256*cpair + 128*i + p, DoubleRow pair-interleaved
    cT2 = (-2.0 * c).T.reshape(2, 2, P, K).transpose(0, 2, 3, 1)
    cT2 = np.ascontiguousarray(cT2).astype(fp8_np)
    c2 = np.einsum("kd,kd->k", c.astype(np.float64), c.astype(np.float64))
    c2 = c2.astype(np.float32)[None, :]                                  # [1, K]
    x2 = np.einsum("nd,nd->n", x.astype(np.float64), x.astype(np.float64))
    x2 = x2.astype(np.float32)                                           # [N]
    m2 = (msk.astype(np.float64) ** 2).astype(np.float32)                # [N]

    in_maps = []
    for i in range(N_CORES):
        sl = slice(i * NS, (i + 1) * NS)
        xT = np.ascontiguousarray(x[sl].T).astype(fp8_np)                # [D, NS]
        x2c = np.ascontiguousarray(x2[sl].reshape(N_TILES, P).T)         # [P, 32]
        m2c = np.ascontiguousarray(m2[sl].reshape(N_TILES, P).T)         # [P, 32]
        in_maps.append({"xT": xT, "cT2": cT2, "c2r": c2, "x2c": x2c, "m2c": m2c,
                        "onesr": np.ones((P, 1), dtype=np.float32)})
    return in_maps


def run(node_repr, mask, centroid_weight, trace=False):
    """Run the device kernel; returns ((graph, node), BassKernelResults)."""
    from concourse.bass_utils import run_bass_kernel_spmd

    in_maps = _prep_in_maps(node_repr, mask, centroid_weight)
    nc = _get_program()
    res = run_bass_kernel_spmd(nc, in_maps, list(range(N_CORES)), trace=trace)

    node_out = np.empty((1, N, K), dtype=np.float32)
    csum = np.zeros(K, dtype=np.float64)
    for i in range(N_CORES):
        node_out[0, i * NS : (i + 1) * NS] = res.results[i]["out"]
        csum += res.results[i]["csum"][0].astype(np.float64)
    msum = float(np.asarray(mask, dtype=np.float64).sum())
    graph = (csum / msum).astype(np.float32)[None, :]  # [1, K]
    return (graph, node_out), res


def kernel(node_repr, mask, centroid_weight):
    (graph, node_out), _ = run(node_repr, mask, centroid_weight, trace=False)
    return graph, node_out
